# revision 4
# baseline (speedup 1.0000x reference)
"""Trainium2 Bass kernel for nn_MultiHeadAttention_90993177133622.

Math (from reference):
  Q = QKV @ Wq.T + bq   (same for K, V)            [B,S,E] -> view [B,S,H,D]
  P[b,s,h,g] = sum_d Q[b,s,h,d] K[b,s,g,d] * sqrt(D)   (per-token [H,H] attn)
  causal tril mask over [H,H], softmax over g
  out = (P @ V).reshape(B,S,E) @ Wo.T + bo

Sharding: data-parallel over the 16384 tokens across 8 cores (2048 each).

v2: engine-balanced attention. The per-token [H,H] attention is pure
elementwise work (PE cannot contract per-token), so it is split across
DVE (fast: fp16 2x) and Pool/GpSimd (slower but otherwise idle), with
ACT doing PSUM->SBUF copies + exp, overlapped with the PE projections.
  - DVE:  QK heads 8..15 as one rectangular mult+tree, softmax, PV heads 4..15
  - Pool: QK heads 0..7 (causal-packed), PV heads 0..3
  - mask: single TT-min against a +/-60000 tile replaces NEG-memset grid
"""
import sys, os, types, ctypes, contextlib, json, math

sys.path.insert(0, "/opt/trn_rl_repo")
import numpy as np

B, S, E, H, D = 4, 4096, 1024, 16, 64
NCORES = 8
TOK = B * S                 # 16384
TPC = TOK // NCORES         # tokens per core: 2048
CHUNK = 128                 # tokens per tile
NCH = TPC // CHUNK          # 16 chunks per core
KC = E // 128               # 8 contraction chunks
SCALE = math.sqrt(D)        # reference MULTIPLIES by sqrt(D)
NEG = -60000.0              # fp16-safe -inf surrogate
BIG = 60000.0


# ---------------------------------------------------------------- infra shims
def _install_ntff_hook():
    """antenv.axon_hooks is missing in this image; provide it so
    run_bass_kernel_spmd(trace=True) can profile via libaxon_pjrt."""
    if "antenv.axon_hooks" in sys.modules:
        return
    mod = types.ModuleType("antenv.axon_hooks")
    state = {"hook": None}
    mod.set_axon_ntff_profile_hook = lambda h: state.__setitem__("hook", h)
    mod.get_axon_ntff_profile_hook = lambda: state["hook"]
    sys.modules["antenv.axon_hooks"] = mod
    try:
        lib = ctypes.CDLL("/opt/axon/libaxon_pjrt.so")
    except OSError:
        return
    if not hasattr(lib, "axon_start_nrt_profile"):
        return
    lib.axon_start_nrt_profile.argtypes = [ctypes.POINTER(ctypes.c_int64), ctypes.c_size_t]
    lib.axon_start_nrt_profile.restype = ctypes.c_int64
    lib.axon_stop_nrt_profile.argtypes = [ctypes.c_char_p]
    lib.axon_stop_nrt_profile.restype = ctypes.c_int64

    @contextlib.contextmanager
    def _hook(output_dir, device_ids):
        import jax
        jax.devices()
        if device_ids:
            ids = (ctypes.c_int64 * len(device_ids))(*device_ids)
            rc = lib.axon_start_nrt_profile(ids, len(device_ids))
        else:
            rc = lib.axon_start_nrt_profile(None, 0)
        if rc != 0:
            raise RuntimeError(f"axon_start_nrt_profile rc={rc}")
        try:
            yield
        finally:
            n = lib.axon_stop_nrt_profile(str(output_dir).encode())
            print(f"profile: {n} file(s) -> {output_dir}", file=sys.stderr)

    mod.set_axon_ntff_profile_hook(_hook)


_install_ntff_hook()

_MAX_WAITS = 2


def _split_waits_json(raw: bytes) -> bytes:
    """This walrus build rejects CTRL instructions with >2 sync waits; split
    extra waits off Drain/EventSemaphore into preceding wait-only ctrls."""
    j = json.loads(raw)
    for fn in j["functions"]:
        for bb in fn["blocks"]:
            out = []
            for inst in bb["instructions"]:
                si = inst.get("sync_info")
                waits = (si or {}).get("on_wait") or []
                limit = _MAX_WAITS if inst.get("opcode") == "EventSemaphore" else 1
                if len(waits) > limit:
                    head, tail = waits[:-limit], waits[-limit:]
                    for k in range(0, len(head), _MAX_WAITS):
                        out.append({
                            "debug": inst.get("debug", 0),
                            "engine": inst["engine"],
                            "ins": [], "outs": [],
                            "name": inst["name"] + f"_w{k}",
                            "opcode": "EventSemaphore",
                            "sync_info": {"on_update": [], "on_wait": head[k:k + _MAX_WAITS]},
                        })
                    si["on_wait"] = tail
                out.append(inst)
            bb["instructions"] = out
    return json.dumps(j).encode()


def _patch_bass(nc):
    orig = nc.to_json_bytes
    nc.to_json_bytes = lambda: _split_waits_json(orig())
    return nc


# ---------------------------------------------------------------- the program
_cache = {}


def _build(is_causal: bool, use_bias: bool = True):
    import concourse.bass as bass
    import concourse.tile as tile
    import concourse.mybir as mybir
    from contextlib import ExitStack

    f32 = mybir.dt.float32
    f16 = mybir.dt.float16
    Alu = mybir.AluOpType
    Act = mybir.ActivationFunctionType

    nc = bass.Bass("TRN2", target_bir_lowering=False, debug=False, enable_asserts=False)

    xt = nc.dram_tensor("xt", [E, TPC], f16, kind="ExternalInput").ap()
    ws = {n: nc.dram_tensor(n, [E, E], f16, kind="ExternalInput").ap()
          for n in ("wq", "wk", "wv", "wo")}
    bias4 = nc.dram_tensor("bias4", [1, 4 * E], f16, kind="ExternalInput").ap()
    ident = nc.dram_tensor("ident", [128, 128], f16, kind="ExternalInput").ap()
    out_d = nc.dram_tensor("out", [TPC, E], f32, kind="ExternalOutput").ap()

    # head split between engines
    PH = 8 if is_causal else 3          # heads on Pool for QK
    DH = H - PH                         # heads on DVE (rectangular)
    # packed pair count for Pool heads (causal: 1+2+..+PH)
    NPP = sum((h + 1) if is_causal else H for h in range(PH))

    def tt(eng, out, in0, in1, op):
        return eng.add_instruction(mybir.InstTensorTensor(
            name=nc.get_next_instruction_name(), op=op,
            ins=[eng.lower_ap(in0), eng.lower_ap(in1)],
            outs=[eng.lower_ap(out)]))

    with tile.TileContext(nc) as tc, ExitStack() as ctx:
        wpool = ctx.enter_context(tc.tile_pool(name="w", bufs=1))
        xpool = ctx.enter_context(tc.tile_pool(name="x", bufs=2))
        qkv = ctx.enter_context(tc.tile_pool(name="qkv", bufs=3))
        gridp = ctx.enter_context(tc.tile_pool(name="grid", bufs=2))
        ppool = ctx.enter_context(tc.tile_pool(name="p", bufs=2))
        stats = ctx.enter_context(tc.tile_pool(name="st", bufs=2))
        qkd = ctx.enter_context(tc.tile_pool(name="qkd", bufs=1))
        qkp = ctx.enter_context(tc.tile_pool(name="qkp", bufs=1))
        pvp = ctx.enter_context(tc.tile_pool(name="pv", bufs=1))
        opool = ctx.enter_context(tc.tile_pool(name="o", bufs=2))
        aop = ctx.enter_context(tc.tile_pool(name="ao", bufs=2))
        o2pool = ctx.enter_context(tc.tile_pool(name="o2", bufs=2))
        psum = ctx.enter_context(tc.tile_pool(name="ps", bufs=2, space="PSUM"))
        pstr = ctx.enter_context(tc.tile_pool(name="pstr", bufs=2, space="PSUM"))
        pso = ctx.enter_context(tc.tile_pool(name="pso", bufs=2, space="PSUM"))

        # ---------------- resident constants
        w_sb = {}
        for n in ("wq", "wk", "wv", "wo"):
            t = wpool.tile([128, KC, E], f16, tag=f"w_{n}", name=f"w_{n}")
            wr = ws[n].rearrange("(k p) n -> p k n", p=128)
            for k in range(KC):
                eng = (nc.sync, nc.gpsimd, nc.scalar)[k % 3]
                eng.dma_start(t[:, k, :], wr[:, k, :])
            w_sb[n] = t
        if use_bias:
            b_sb = wpool.tile([1, 4 * E], f16, tag="bias")
            nc.sync.dma_start(b_sb[:], bias4[:])
            ones = wpool.tile([1, CHUNK], f16, tag="ones")
            nc.vector.memset(ones[:], 1.0)
        id_sb = wpool.tile([128, 128], f16, tag="ident")
        nc.sync.dma_start(id_sb[:], ident[:])
        # causal mask tile: +BIG on valid (g<=h), NEG on invalid; TT-min
        # against it masks the logit grid in one op
        mceil = wpool.tile([128, H * H], f16, tag="mceil")
        nc.gpsimd.memset(mceil[:], BIG)
        if is_causal:
            for h in range(H - 1):
                nc.gpsimd.memset(mceil[:, h * H + h + 1: (h + 1) * H], NEG)

        xt_r = xt.rearrange("(k p) t -> p k t", p=128)

        # ---------------- per-chunk stage issue helpers
        state = {}

        def issue_proj(ci):
            tsl = slice(ci * CHUNK, (ci + 1) * CHUNK)
            x_sb = xpool.tile([128, KC, CHUNK], f16, tag="x")
            nc.sync.dma_start(x_sb[:], xt_r[:, :, tsl])
            sb = {}
            for pi, n in enumerate(("wq", "wk", "wv")):
                ps = psum.tile([128, E], f32, tag="proj")
                for k in range(KC):
                    for nn in range(2):
                        nsl = slice(nn * 512, (nn + 1) * 512)
                        nc.tensor.matmul(
                            ps[:, nsl],
                            x_sb[:, k, :],
                            w_sb[n][:, k, nsl],
                            start=(k == 0),
                            stop=(k == KC - 1 and not use_bias))
                if use_bias:
                    for nn in range(2):
                        nsl = slice(nn * 512, (nn + 1) * 512)
                        nc.tensor.matmul(
                            ps[:, nsl],
                            ones[:],
                            b_sb[0:1, pi * E + nn * 512: pi * E + (nn + 1) * 512],
                            start=False, stop=True)
                t = qkv.tile([128, E], f16, tag=n, name=n)
                nc.scalar.copy(t[:], ps[:])
                sb[n] = t
            state[ci] = {"q": sb["wq"], "k": sb["wk"], "v": sb["wv"],
                         "x": x_sb, "tsl": tsl}

        def issue_qk(ci):
            st = state[ci]
            q_sb, k_sb = st["q"], st["k"]
            grid = gridp.tile([128, H * H], f16, tag="grid")
            k_v = k_sb[:].rearrange("p (g d) -> p g d", g=H)

            # --- Pool: heads 0..PH-1, causal-packed products + tree + finals
            gmax = (lambda h: h + 1) if is_causal else (lambda h: H)
            off = [0] * (PH + 1)
            for h in range(PH):
                off[h + 1] = off[h] + gmax(h)
            qp_p = qkp.tile([128, NPP * D], f16, tag="qpp")
            for h in range(PH):
                g = gmax(h)
                tt(nc.gpsimd,
                   qp_p[:, off[h] * D:off[h + 1] * D].rearrange("p (g d) -> p g d", g=g),
                   q_sb[:, h * D:(h + 1) * D].unsqueeze(1).broadcast_to([128, g, D]),
                   k_v[:, :g, :], Alu.mult)
            lv = qp_p[:].rearrange("p (n d) -> p n d", n=NPP)
            w = D
            li = 0
            while w > 2:
                w //= 2
                nt = qkp.tile([128, NPP * w], f16, tag=f"ptr{li}", name=f"ptr{li}")
                tt(nc.gpsimd,
                   nt[:].rearrange("p (n d) -> p n d", n=NPP),
                   lv[:, :, 0:w], lv[:, :, w:2 * w], Alu.add)
                lv = nt[:].rearrange("p (n d) -> p n d", n=NPP)
                li += 1
            for h in range(PH):
                g = gmax(h)
                tt(nc.gpsimd,
                   grid[:, h * H:h * H + g].unsqueeze(2),
                   lv[:, off[h]:off[h + 1], 0:1], lv[:, off[h]:off[h + 1], 1:2],
                   Alu.add)

            # --- DVE: heads PH..15, one rectangle [DH, H, D] + tree
            qp_d = qkd.tile([128, DH * H * D], f16, tag="qpd")
            tt(nc.vector,
               qp_d[:].rearrange("p (h g d) -> p h g d", h=DH, g=H),
               q_sb[:, PH * D:].rearrange("p (h d) -> p h d", h=DH)
               .unsqueeze(2).broadcast_to([128, DH, H, D]),
               k_v[:].unsqueeze(1).broadcast_to([128, DH, H, D]),
               Alu.mult)
            n2 = DH * H
            lv = qp_d[:].rearrange("p (n d) -> p n d", n=n2)
            w = D
            li = 0
            while w > 2:
                w //= 2
                nt = qkd.tile([128, n2 * w], f16, tag=f"dtr{li}", name=f"dtr{li}")
                tt(nc.vector,
                   nt[:].rearrange("p (n d) -> p n d", n=n2),
                   lv[:, :, 0:w], lv[:, :, w:2 * w], Alu.add)
                lv = nt[:].rearrange("p (n d) -> p n d", n=n2)
                li += 1
            # final level writes the grid rows PH..15 compactly
            tt(nc.vector,
               grid[:, PH * H:].rearrange("p (n o) -> p n o", n=n2),
               lv[:, :, 0:1], lv[:, :, 1:2], Alu.add)
            st["grid"] = grid

        def issue_softmax(ci):
            st = state[ci]
            grid = st["grid"]
            gm = gridp.tile([128, H * H], f16, tag="gm")
            tt(nc.vector, gm[:], grid[:], mceil[:], Alu.min)
            g3 = gm[:].rearrange("p (h g) -> p h g", h=H)
            mx = stats.tile([128, H], f16, tag="mx")
            nc.vector.tensor_reduce(mx[:], g3, mybir.AxisListType.X, Alu.max)
            p2 = ppool.tile([128, H * H], f16, tag="psub")
            tt(nc.vector,
               p2[:].rearrange("p (h g) -> p h g", h=H),
               g3, mx[:].unsqueeze(2).broadcast_to([128, H, H]), Alu.subtract)
            ex = ppool.tile([128, H * H], f16, tag="pexp")
            nc.scalar.activation(ex[:], p2[:], Act.Exp, scale=float(SCALE))
            sm = stats.tile([128, H], f32, tag="sm")
            nc.vector.tensor_reduce(
                sm[:], ex[:].rearrange("p (h g) -> p h g", h=H),
                mybir.AxisListType.X, Alu.add)
            rc = stats.tile([128, H], f32, tag="rc")
            nc.vector.reciprocal(rc[:], sm[:])
            rc16 = stats.tile([128, H], f16, tag="rc16")
            nc.scalar.copy(rc16[:], rc[:])
            pn = ppool.tile([128, H * H], f16, tag="pnorm")
            tt(nc.vector,
               pn[:].rearrange("p (h g) -> p h g", h=H),
               ex[:].rearrange("p (h g) -> p h g", h=H),
               rc16[:].unsqueeze(2).broadcast_to([128, H, H]), Alu.mult)
            st["pn"] = pn

        def _pv_group(eng, st, h0, h1, Gp, tagc):
            pn, v_sb = st["pn"], st["v"]
            attn = st["attn"]
            v_dg = v_sb[:].rearrange("p (d g) -> p d g", g=H)  # V is d-major
            nh = h1 - h0
            gp = pvp.tile([128, nh * D * Gp], f16, tag=f"gp{tagc}", name=f"gp{tagc}")
            tt(eng,
               gp[:].rearrange("p (h d g) -> p h d g", h=nh, d=D),
               pn[:, h0 * H:h1 * H]
               .rearrange("p (h g) -> p h g", h=nh)[:, :, :Gp]
               .unsqueeze(2).broadcast_to([128, nh, D, Gp]),
               v_dg[:, :, :Gp].unsqueeze(1)
               .broadcast_to([128, nh, D, Gp]),
               Alu.mult)
            lvv = gp[:].rearrange("p (n g) -> p n g", n=nh * D)
            w2 = Gp
            li = 0
            while w2 > 2:
                w2 //= 2
                nt2 = pvp.tile([128, nh * D * w2], f16,
                               tag=f"pt{tagc}{li}", name=f"pt{tagc}{li}")
                tt(eng,
                   nt2[:].rearrange("p (n g) -> p n g", n=nh * D),
                   lvv[:, :, 0:w2], lvv[:, :, w2:2 * w2], Alu.add)
                lvv = nt2[:].rearrange("p (n g) -> p n g", n=nh * D)
                li += 1
            tt(eng,
               attn[:, h0 * D:h1 * D].unsqueeze(2),
               lvv[:, :, 0:1], lvv[:, :, 1:2], Alu.add)

        def issue_pv(ci):
            st = state[ci]
            attn = opool.tile([128, E], f16, tag="attn")
            st["attn"] = attn
            with nc.allow_low_precision("fp16 attn accumulation over 16 heads"):
                if is_causal:
                    _pv_group(nc.gpsimd, st, 0, 4, 4, "a")
                    _pv_group(nc.vector, st, 4, 8, 8, "b")
                    _pv_group(nc.vector, st, 8, 16, 16, "c")
                else:
                    _pv_group(nc.gpsimd, st, 0, 3, 16, "n")
                    _pv_group(nc.vector, st, 3, 16, 16, "m")

        def issue_out(ci):
            st = state[ci]
            attn, tsl = st["attn"], st["tsl"]
            ps_t = pstr.tile([128, E], f16, tag="tr")
            for j in range(KC):
                nc.tensor.transpose(
                    ps_t[:, j * 128:(j + 1) * 128],
                    attn[:, j * 128:(j + 1) * 128], id_sb[:])
            ao = aop.tile([128, KC, CHUNK], f16, tag="ao")
            nc.scalar.copy(ao[:], ps_t[:])
            o_sb = o2pool.tile([128, E], f32, tag="out", name="o_sb")
            for nn in range(2):
                nsl = slice(nn * 512, (nn + 1) * 512)
                ps_o = pso.tile([128, 512], f32, tag="oproj")
                for k in range(KC):
                    nc.tensor.matmul(
                        ps_o[:],
                        ao[:, k, :],
                        w_sb["wo"][:, k, nsl],
                        start=(k == 0),
                        stop=(k == KC - 1 and not use_bias))
                if use_bias:
                    nc.tensor.matmul(
                        ps_o[:],
                        ones[:],
                        b_sb[0:1, 3 * E + nn * 512: 3 * E + (nn + 1) * 512],
                        start=False, stop=True)
                nc.scalar.copy(o_sb[:, nsl], ps_o[:])
            nc.sync.dma_start(out_d[tsl, :], o_sb[:])
            del state[ci]

        # ---------------- software-pipelined issue order
        # softmax/pv(ci-1) first: their DVE/ACT/Pool ops are ready at the top
        # of the iteration; proj(ci) then refills PE; qk(ci) lands when the
        # q/k copies arrive; out(ci-1) uses PE after proj(ci).
        for ci in range(NCH + 1):
            if ci >= 1:
                issue_softmax(ci - 1)
                issue_pv(ci - 1)
            if ci < NCH:
                issue_proj(ci)
                issue_qk(ci)
            if ci >= 1:
                issue_out(ci - 1)

    return _patch_bass(nc)


LAST_RESULTS = None


def kernel(**inputs) -> np.ndarray:
    global LAST_RESULTS
    from concourse import bass_utils

    qkv = np.asarray(inputs["QKV"], dtype=np.float32)
    is_causal = bool(int(np.asarray(inputs["is_causal"])))
    X = np.ascontiguousarray(qkv.reshape(TOK, E).astype(np.float16))
    wts = {n: np.ascontiguousarray(np.asarray(inputs[wn], dtype=np.float32).T.astype(np.float16))
           for n, wn in (("wq", "Wq"), ("wk", "Wk"), ("wv", "Wv"), ("wo", "Wo"))}
    # V projection emits d-major head layout: col d*16+g holds head g, dim d
    wts["wv"] = np.ascontiguousarray(
        wts["wv"].reshape(E, H, D).transpose(0, 2, 1).reshape(E, E))
    bias4 = np.ascontiguousarray(np.concatenate([
        np.asarray(inputs[b], dtype=np.float32) for b in ("bq", "bk", "bv", "bo")])[None, :].astype(np.float16))
    ident = np.eye(128, dtype=np.float16)

    use_bias = any(
        float(np.abs(np.asarray(inputs[b])).max()) != 0.0
        for b in ("bq", "bk", "bv", "bo"))
    key = (is_causal, use_bias)
    if key not in _cache:
        _cache[key] = _build(is_causal, use_bias)
    nc = _cache[key]

    in_maps = []
    for c in range(NCORES):
        xt_c = np.ascontiguousarray(X[c * TPC:(c + 1) * TPC].T)
        in_maps.append({"xt": xt_c, "bias4": bias4, "ident": ident, **wts})

    trace = bool(int(os.environ.get("BASSMHA_TRACE", "0")))
    res = bass_utils.run_bass_kernel_spmd(
        nc, in_maps, core_ids=list(range(NCORES)), trace=trace)
    LAST_RESULTS = res
    out = np.concatenate([res.results[c]["out"] for c in range(NCORES)], axis=0)
    return out.reshape(B, S, E)


if __name__ == "__main__":
    np.random.seed(0)
    fake = {
        "QKV": np.random.randn(B, S, E).astype(np.float32),
        "Wq": np.random.randn(E, E).astype(np.float32) * 0.02,
        "bq": np.zeros(E, np.float32),
        "Wk": np.random.randn(E, E).astype(np.float32) * 0.02,
        "bk": np.zeros(E, np.float32),
        "Wv": np.random.randn(E, E).astype(np.float32) * 0.02,
        "bv": np.zeros(E, np.float32),
        "Wo": np.random.randn(E, E).astype(np.float32) * 0.02,
        "bo": np.zeros(E, np.float32),
        "is_causal": 1,
    }
    o = kernel(**fake)
    print("kernel ok", o.shape, o.dtype, float(np.abs(o).mean()))


# revision 9
# speedup vs baseline: 1.0010x; 1.0010x over previous
"""Trainium2 Bass kernel for nn_MultiHeadAttention_90993177133622.

Math (from reference):
  Q = QKV @ Wq.T + bq   (same for K, V)            [B,S,E] -> view [B,S,H,D]
  P[b,s,h,g] = sum_d Q[b,s,h,d] K[b,s,g,d] * sqrt(D)   (per-token [H,H] attn)
  causal tril mask over [H,H], softmax over g
  out = (P @ V).reshape(B,S,E) @ Wo.T + bo

Sharding: data-parallel over the 16384 tokens across 8 cores (2048 each).

v2: engine-balanced attention. The per-token [H,H] attention is pure
elementwise work (PE cannot contract per-token), so it is split across
DVE (fast: fp16 2x) and Pool/GpSimd (slower but otherwise idle), with
ACT doing PSUM->SBUF copies + exp, overlapped with the PE projections.
  - DVE:  QK heads 8..15 as one rectangular mult+tree, softmax, PV heads 4..15
  - Pool: QK heads 0..7 (causal-packed), PV heads 0..3
  - mask: single TT-min against a +/-60000 tile replaces NEG-memset grid
"""
import sys, os, types, ctypes, contextlib, json, math

sys.path.insert(0, "/opt/trn_rl_repo")
import numpy as np

B, S, E, H, D = 4, 4096, 1024, 16, 64
NCORES = 8
TOK = B * S                 # 16384
TPC = TOK // NCORES         # tokens per core: 2048
CHUNK = 128                 # tokens per tile
NCH = TPC // CHUNK          # 16 chunks per core
KC = E // 128               # 8 contraction chunks
SCALE = math.sqrt(D)        # reference MULTIPLIES by sqrt(D)
NEG = -60000.0              # fp16-safe -inf surrogate
BIG = 60000.0


# ---------------------------------------------------------------- infra shims
def _install_ntff_hook():
    """antenv.axon_hooks is missing in this image; provide it so
    run_bass_kernel_spmd(trace=True) can profile via libaxon_pjrt."""
    if "antenv.axon_hooks" in sys.modules:
        return
    mod = types.ModuleType("antenv.axon_hooks")
    state = {"hook": None}
    mod.set_axon_ntff_profile_hook = lambda h: state.__setitem__("hook", h)
    mod.get_axon_ntff_profile_hook = lambda: state["hook"]
    sys.modules["antenv.axon_hooks"] = mod
    try:
        lib = ctypes.CDLL("/opt/axon/libaxon_pjrt.so")
    except OSError:
        return
    if not hasattr(lib, "axon_start_nrt_profile"):
        return
    lib.axon_start_nrt_profile.argtypes = [ctypes.POINTER(ctypes.c_int64), ctypes.c_size_t]
    lib.axon_start_nrt_profile.restype = ctypes.c_int64
    lib.axon_stop_nrt_profile.argtypes = [ctypes.c_char_p]
    lib.axon_stop_nrt_profile.restype = ctypes.c_int64

    @contextlib.contextmanager
    def _hook(output_dir, device_ids):
        import jax
        jax.devices()
        if device_ids:
            ids = (ctypes.c_int64 * len(device_ids))(*device_ids)
            rc = lib.axon_start_nrt_profile(ids, len(device_ids))
        else:
            rc = lib.axon_start_nrt_profile(None, 0)
        if rc != 0:
            raise RuntimeError(f"axon_start_nrt_profile rc={rc}")
        try:
            yield
        finally:
            n = lib.axon_stop_nrt_profile(str(output_dir).encode())
            print(f"profile: {n} file(s) -> {output_dir}", file=sys.stderr)

    mod.set_axon_ntff_profile_hook(_hook)


_install_ntff_hook()

_MAX_WAITS = 2


def _split_waits_json(raw: bytes) -> bytes:
    """This walrus build rejects CTRL instructions with >2 sync waits; split
    extra waits off Drain/EventSemaphore into preceding wait-only ctrls."""
    j = json.loads(raw)
    for fn in j["functions"]:
        for bb in fn["blocks"]:
            out = []
            for inst in bb["instructions"]:
                si = inst.get("sync_info")
                waits = (si or {}).get("on_wait") or []
                limit = _MAX_WAITS if inst.get("opcode") == "EventSemaphore" else 1
                if len(waits) > limit:
                    head, tail = waits[:-limit], waits[-limit:]
                    for k in range(0, len(head), _MAX_WAITS):
                        out.append({
                            "debug": inst.get("debug", 0),
                            "engine": inst["engine"],
                            "ins": [], "outs": [],
                            "name": inst["name"] + f"_w{k}",
                            "opcode": "EventSemaphore",
                            "sync_info": {"on_update": [], "on_wait": head[k:k + _MAX_WAITS]},
                        })
                    si["on_wait"] = tail
                out.append(inst)
            bb["instructions"] = out
    return json.dumps(j).encode()


def _patch_bass(nc):
    orig = nc.to_json_bytes
    nc.to_json_bytes = lambda: _split_waits_json(orig())
    return nc


# ---------------------------------------------------------------- the program
_cache = {}


def _build(is_causal: bool, use_bias: bool = True):
    import concourse.bass as bass
    import concourse.tile as tile
    import concourse.mybir as mybir
    from contextlib import ExitStack

    f32 = mybir.dt.float32
    f16 = mybir.dt.float16
    Alu = mybir.AluOpType
    Act = mybir.ActivationFunctionType

    nc = bass.Bass("TRN2", target_bir_lowering=False, debug=False, enable_asserts=False)

    xt = nc.dram_tensor("xt", [E, TPC], f16, kind="ExternalInput").ap()
    ws = {n: nc.dram_tensor(n, [E, E], f16, kind="ExternalInput").ap()
          for n in ("wq", "wk", "wv", "wo")}
    bias4 = nc.dram_tensor("bias4", [1, 4 * E], f16, kind="ExternalInput").ap()
    ident = nc.dram_tensor("ident", [128, 128], f16, kind="ExternalInput").ap()
    out_d = nc.dram_tensor("out", [TPC, E], f32, kind="ExternalOutput").ap()

    def tt(eng, out, in0, in1, op):
        return eng.add_instruction(mybir.InstTensorTensor(
            name=nc.get_next_instruction_name(), op=op,
            ins=[eng.lower_ap(in0), eng.lower_ap(in1)],
            outs=[eng.lower_ap(out)]))

    with tile.TileContext(nc) as tc, ExitStack() as ctx:
        wpool = ctx.enter_context(tc.tile_pool(name="w", bufs=1))
        xpool = ctx.enter_context(tc.tile_pool(name="x", bufs=2))
        qkv = ctx.enter_context(tc.tile_pool(name="qkv", bufs=3))
        gridp = ctx.enter_context(tc.tile_pool(name="grid", bufs=2))
        ppool = ctx.enter_context(tc.tile_pool(name="p", bufs=2))
        stats = ctx.enter_context(tc.tile_pool(name="st", bufs=2))
        qkd = ctx.enter_context(tc.tile_pool(name="qkd", bufs=1))
        pvp = ctx.enter_context(tc.tile_pool(name="pv", bufs=1))
        opool = ctx.enter_context(tc.tile_pool(name="o", bufs=2))
        aop = ctx.enter_context(tc.tile_pool(name="ao", bufs=2))
        o2pool = ctx.enter_context(tc.tile_pool(name="o2", bufs=2))
        psum = ctx.enter_context(tc.tile_pool(name="ps", bufs=2, space="PSUM"))
        pstr = ctx.enter_context(tc.tile_pool(name="pstr", bufs=2, space="PSUM"))
        pso = ctx.enter_context(tc.tile_pool(name="pso", bufs=2, space="PSUM"))

        # ---------------- resident constants
        w_sb = {}
        for n in ("wq", "wk", "wv", "wo"):
            t = wpool.tile([128, KC, E], f16, tag=f"w_{n}", name=f"w_{n}")
            wr = ws[n].rearrange("(k p) n -> p k n", p=128)
            for k in range(KC):
                eng = (nc.sync, nc.gpsimd, nc.scalar)[k % 3]
                eng.dma_start(t[:, k, :], wr[:, k, :])
            w_sb[n] = t
        if use_bias:
            b_sb = wpool.tile([1, 4 * E], f16, tag="bias")
            nc.sync.dma_start(b_sb[:], bias4[:])
            ones = wpool.tile([1, CHUNK], f16, tag="ones")
            nc.vector.memset(ones[:], 1.0)
        id_sb = wpool.tile([128, 128], f16, tag="ident")
        nc.sync.dma_start(id_sb[:], ident[:])

        xt_r = xt.rearrange("(k p) t -> p k t", p=128)

        # ---------------- per-chunk stage issue helpers
        state = {}

        def issue_proj(ci):
            tsl = slice(ci * CHUNK, (ci + 1) * CHUNK)
            x_sb = xpool.tile([128, KC, CHUNK], f16, tag="x")
            nc.sync.dma_start(x_sb[:], xt_r[:, :, tsl])
            sb = {}
            for pi, n in enumerate(("wq", "wk", "wv")):
                ps = psum.tile([128, E], f32, tag="proj")
                for k in range(KC):
                    for nn in range(2):
                        nsl = slice(nn * 512, (nn + 1) * 512)
                        nc.tensor.matmul(
                            ps[:, nsl],
                            x_sb[:, k, :],
                            w_sb[n][:, k, nsl],
                            start=(k == 0),
                            stop=(k == KC - 1 and not use_bias))
                if use_bias:
                    for nn in range(2):
                        nsl = slice(nn * 512, (nn + 1) * 512)
                        nc.tensor.matmul(
                            ps[:, nsl],
                            ones[:],
                            b_sb[0:1, pi * E + nn * 512: pi * E + (nn + 1) * 512],
                            start=False, stop=True)
                t = qkv.tile([128, E], f16, tag=n, name=n)
                nc.scalar.copy(t[:], ps[:])
                sb[n] = t
            state[ci] = {"q": sb["wq"], "k": sb["wk"], "v": sb["wv"],
                         "x": x_sb, "tsl": tsl}

        def issue_qk(ci):
            st = state[ci]
            q_sb, k_sb = st["q"], st["k"]
            grid = gridp.tile([128, H * H], f16, tag="grid")
            nc.gpsimd.memset(grid[:], NEG)
            k_v = k_sb[:].rearrange("p (g d) -> p g d", g=H)

            # --- DVE: all heads causal-packed: products + tree + finals
            gmax = (lambda h: h + 1) if is_causal else (lambda h: H)
            NP = sum(gmax(h) for h in range(H))
            off = [0] * (H + 1)
            for h in range(H):
                off[h + 1] = off[h] + gmax(h)
            qp_d = qkd.tile([128, NP * D], f16, tag="qpd")
            for h in range(H):
                g = gmax(h)
                tt(nc.vector,
                   qp_d[:, off[h] * D:off[h + 1] * D].rearrange("p (g d) -> p g d", g=g),
                   q_sb[:, h * D:(h + 1) * D].unsqueeze(1).broadcast_to([128, g, D]),
                   k_v[:, :g, :], Alu.mult)
            lv = qp_d[:].rearrange("p (n d) -> p n d", n=NP)
            w = D
            li = 0
            while w > 2:
                w //= 2
                nt = qkd.tile([128, NP * w], f16, tag=f"dtr{li}", name=f"dtr{li}")
                tt(nc.vector,
                   nt[:].rearrange("p (n d) -> p n d", n=NP),
                   lv[:, :, 0:w], lv[:, :, w:2 * w], Alu.add)
                lv = nt[:].rearrange("p (n d) -> p n d", n=NP)
                li += 1
            for h in range(H):
                g = gmax(h)
                tt(nc.vector,
                   grid[:, h * H:h * H + g].unsqueeze(2),
                   lv[:, off[h]:off[h + 1], 0:1], lv[:, off[h]:off[h + 1], 1:2],
                   Alu.add)
            st["grid"] = grid

        def issue_softmax(ci):
            st = state[ci]
            grid = st["grid"]
            g3 = grid[:].rearrange("p (h g) -> p h g", h=H)
            mx = stats.tile([128, H], f16, tag="mx")
            nc.vector.tensor_reduce(mx[:], g3, mybir.AxisListType.X, Alu.max)
            p2 = ppool.tile([128, H * H], f16, tag="psub")
            tt(nc.vector,
               p2[:].rearrange("p (h g) -> p h g", h=H),
               g3, mx[:].unsqueeze(2).broadcast_to([128, H, H]), Alu.subtract)
            ex = ppool.tile([128, H * H], f16, tag="pexp")
            nc.scalar.activation(ex[:], p2[:], Act.Exp, scale=float(SCALE))
            sm = stats.tile([128, H], f32, tag="sm")
            nc.vector.tensor_reduce(
                sm[:], ex[:].rearrange("p (h g) -> p h g", h=H),
                mybir.AxisListType.X, Alu.add)
            rc = stats.tile([128, H], f32, tag="rc")
            nc.vector.reciprocal(rc[:], sm[:])
            rc16 = stats.tile([128, H], f16, tag="rc16")
            nc.scalar.copy(rc16[:], rc[:])
            pn = ppool.tile([128, H * H], f16, tag="pnorm")
            tt(nc.vector,
               pn[:].rearrange("p (h g) -> p h g", h=H),
               ex[:].rearrange("p (h g) -> p h g", h=H),
               rc16[:].unsqueeze(2).broadcast_to([128, H, H]), Alu.mult)
            st["pn"] = pn

        def _pv_group(eng, st, h0, h1, Gp, tagc):
            pn, v_sb = st["pn"], st["v"]
            attn = st["attn"]
            v_dg = v_sb[:].rearrange("p (d g) -> p d g", g=H)  # V is d-major
            nh = h1 - h0
            gp = pvp.tile([128, nh * D * Gp], f16, tag=f"gp{tagc}", name=f"gp{tagc}")
            tt(eng,
               gp[:].rearrange("p (h d g) -> p h d g", h=nh, d=D),
               pn[:, h0 * H:h1 * H]
               .rearrange("p (h g) -> p h g", h=nh)[:, :, :Gp]
               .unsqueeze(2).broadcast_to([128, nh, D, Gp]),
               v_dg[:, :, :Gp].unsqueeze(1)
               .broadcast_to([128, nh, D, Gp]),
               Alu.mult)
            lvv = gp[:].rearrange("p (n g) -> p n g", n=nh * D)
            w2 = Gp
            li = 0
            while w2 > 2:
                w2 //= 2
                nt2 = pvp.tile([128, nh * D * w2], f16,
                               tag=f"pt{tagc}{li}", name=f"pt{tagc}{li}")
                tt(eng,
                   nt2[:].rearrange("p (n g) -> p n g", n=nh * D),
                   lvv[:, :, 0:w2], lvv[:, :, w2:2 * w2], Alu.add)
                lvv = nt2[:].rearrange("p (n g) -> p n g", n=nh * D)
                li += 1
            tt(eng,
               attn[:, h0 * D:h1 * D].unsqueeze(2),
               lvv[:, :, 0:1], lvv[:, :, 1:2], Alu.add)

        def issue_pv(ci):
            st = state[ci]
            attn = opool.tile([128, E], f16, tag="attn")
            st["attn"] = attn
            with nc.allow_low_precision("fp16 attn accumulation over 16 heads"):
                if is_causal:
                    _pv_group(nc.gpsimd, st, 0, 8, 8, "a")
                    _pv_group(nc.vector, st, 8, 16, 16, "c")
                else:
                    _pv_group(nc.gpsimd, st, 0, 5, 16, "n")
                    _pv_group(nc.vector, st, 5, 16, 16, "m")

        def issue_out(ci):
            st = state[ci]
            attn, tsl = st["attn"], st["tsl"]
            ps_t = pstr.tile([128, E], f16, tag="tr")
            for j in range(KC):
                nc.tensor.transpose(
                    ps_t[:, j * 128:(j + 1) * 128],
                    attn[:, j * 128:(j + 1) * 128], id_sb[:])
            ao = aop.tile([128, KC, CHUNK], f16, tag="ao")
            nc.scalar.copy(ao[:], ps_t[:])
            o_sb = o2pool.tile([128, E], f32, tag="out", name="o_sb")
            for nn in range(2):
                nsl = slice(nn * 512, (nn + 1) * 512)
                ps_o = pso.tile([128, 512], f32, tag="oproj")
                for k in range(KC):
                    nc.tensor.matmul(
                        ps_o[:],
                        ao[:, k, :],
                        w_sb["wo"][:, k, nsl],
                        start=(k == 0),
                        stop=(k == KC - 1 and not use_bias))
                if use_bias:
                    nc.tensor.matmul(
                        ps_o[:],
                        ones[:],
                        b_sb[0:1, 3 * E + nn * 512: 3 * E + (nn + 1) * 512],
                        start=False, stop=True)
                nc.scalar.copy(o_sb[:, nsl], ps_o[:])
            nc.sync.dma_start(out_d[tsl, :], o_sb[:])
            del state[ci]

        # ---------------- software-pipelined issue order
        # softmax/pv(ci-1) first: their DVE/ACT/Pool ops are ready at the top
        # of the iteration; proj(ci) then refills PE; qk(ci) lands when the
        # q/k copies arrive; out(ci-1) uses PE after proj(ci).
        for ci in range(NCH + 1):
            if ci >= 1:
                issue_softmax(ci - 1)
                issue_pv(ci - 1)
            if ci < NCH:
                issue_proj(ci)
                issue_qk(ci)
            if ci >= 1:
                issue_out(ci - 1)

    return _patch_bass(nc)


LAST_RESULTS = None


def kernel(**inputs) -> np.ndarray:
    global LAST_RESULTS
    from concourse import bass_utils

    qkv = np.asarray(inputs["QKV"], dtype=np.float32)
    is_causal = bool(int(np.asarray(inputs["is_causal"])))
    X = np.ascontiguousarray(qkv.reshape(TOK, E).astype(np.float16))
    wts = {n: np.ascontiguousarray(np.asarray(inputs[wn], dtype=np.float32).T.astype(np.float16))
           for n, wn in (("wq", "Wq"), ("wk", "Wk"), ("wv", "Wv"), ("wo", "Wo"))}
    # V projection emits d-major head layout: col d*16+g holds head g, dim d
    wts["wv"] = np.ascontiguousarray(
        wts["wv"].reshape(E, H, D).transpose(0, 2, 1).reshape(E, E))
    bias4 = np.ascontiguousarray(np.concatenate([
        np.asarray(inputs[b], dtype=np.float32) for b in ("bq", "bk", "bv", "bo")])[None, :].astype(np.float16))
    ident = np.eye(128, dtype=np.float16)

    use_bias = any(
        float(np.abs(np.asarray(inputs[b])).max()) != 0.0
        for b in ("bq", "bk", "bv", "bo"))
    key = (is_causal, use_bias)
    if key not in _cache:
        _cache[key] = _build(is_causal, use_bias)
    nc = _cache[key]

    in_maps = []
    for c in range(NCORES):
        xt_c = np.ascontiguousarray(X[c * TPC:(c + 1) * TPC].T)
        in_maps.append({"xt": xt_c, "bias4": bias4, "ident": ident, **wts})

    trace = bool(int(os.environ.get("BASSMHA_TRACE", "0")))
    res = bass_utils.run_bass_kernel_spmd(
        nc, in_maps, core_ids=list(range(NCORES)), trace=trace)
    LAST_RESULTS = res
    out = np.concatenate([res.results[c]["out"] for c in range(NCORES)], axis=0)
    return out.reshape(B, S, E)


if __name__ == "__main__":
    np.random.seed(0)
    fake = {
        "QKV": np.random.randn(B, S, E).astype(np.float32),
        "Wq": np.random.randn(E, E).astype(np.float32) * 0.02,
        "bq": np.zeros(E, np.float32),
        "Wk": np.random.randn(E, E).astype(np.float32) * 0.02,
        "bk": np.zeros(E, np.float32),
        "Wv": np.random.randn(E, E).astype(np.float32) * 0.02,
        "bv": np.zeros(E, np.float32),
        "Wo": np.random.randn(E, E).astype(np.float32) * 0.02,
        "bo": np.zeros(E, np.float32),
        "is_causal": 1,
    }
    o = kernel(**fake)
    print("kernel ok", o.shape, o.dtype, float(np.abs(o).mean()))


# revision 11
# speedup vs baseline: 1.4501x; 1.4486x over previous
"""Trainium2 Bass kernel for nn_MultiHeadAttention_90993177133622.

Math (from reference):
  Q = QKV @ Wq.T + bq   (same for K, V)            [B,S,E] -> view [B,S,H,D]
  P[b,s,h,g] = sum_d Q[b,s,h,d] K[b,s,g,d] * sqrt(D)   (per-token [H,H] attn)
  causal tril mask over [H,H], softmax over g
  out = (P @ V).reshape(B,S,E) @ Wo.T + bo

Sharding: data-parallel over the 16384 tokens across 8 cores (2048 each).

v2: engine-balanced attention. The per-token [H,H] attention is pure
elementwise work (PE cannot contract per-token), so it is split across
DVE (fast: fp16 2x) and Pool/GpSimd (slower but otherwise idle), with
ACT doing PSUM->SBUF copies + exp, overlapped with the PE projections.
  - DVE:  QK heads 8..15 as one rectangular mult+tree, softmax, PV heads 4..15
  - Pool: QK heads 0..7 (causal-packed), PV heads 0..3
  - mask: single TT-min against a +/-60000 tile replaces NEG-memset grid
"""
import sys, os, types, ctypes, contextlib, json, math

sys.path.insert(0, "/opt/trn_rl_repo")
import numpy as np

B, S, E, H, D = 4, 4096, 1024, 16, 64
NCORES = 8
TOK = B * S                 # 16384
TPC = TOK // NCORES         # tokens per core: 2048
CHUNK = 128                 # tokens per tile
NCH = TPC // CHUNK          # 16 chunks per core
KC = E // 128               # 8 contraction chunks
SCALE = math.sqrt(D)        # reference MULTIPLIES by sqrt(D)
NEG = -60000.0              # fp16-safe -inf surrogate
BIG = 60000.0


# ---------------------------------------------------------------- infra shims
def _install_ntff_hook():
    """antenv.axon_hooks is missing in this image; provide it so
    run_bass_kernel_spmd(trace=True) can profile via libaxon_pjrt."""
    if "antenv.axon_hooks" in sys.modules:
        return
    mod = types.ModuleType("antenv.axon_hooks")
    state = {"hook": None}
    mod.set_axon_ntff_profile_hook = lambda h: state.__setitem__("hook", h)
    mod.get_axon_ntff_profile_hook = lambda: state["hook"]
    sys.modules["antenv.axon_hooks"] = mod
    try:
        lib = ctypes.CDLL("/opt/axon/libaxon_pjrt.so")
    except OSError:
        return
    if not hasattr(lib, "axon_start_nrt_profile"):
        return
    lib.axon_start_nrt_profile.argtypes = [ctypes.POINTER(ctypes.c_int64), ctypes.c_size_t]
    lib.axon_start_nrt_profile.restype = ctypes.c_int64
    lib.axon_stop_nrt_profile.argtypes = [ctypes.c_char_p]
    lib.axon_stop_nrt_profile.restype = ctypes.c_int64

    @contextlib.contextmanager
    def _hook(output_dir, device_ids):
        import jax
        jax.devices()
        if device_ids:
            ids = (ctypes.c_int64 * len(device_ids))(*device_ids)
            rc = lib.axon_start_nrt_profile(ids, len(device_ids))
        else:
            rc = lib.axon_start_nrt_profile(None, 0)
        if rc != 0:
            raise RuntimeError(f"axon_start_nrt_profile rc={rc}")
        try:
            yield
        finally:
            n = lib.axon_stop_nrt_profile(str(output_dir).encode())
            print(f"profile: {n} file(s) -> {output_dir}", file=sys.stderr)

    mod.set_axon_ntff_profile_hook(_hook)


_install_ntff_hook()

_MAX_WAITS = 2


def _split_waits_json(raw: bytes) -> bytes:
    """This walrus build rejects CTRL instructions with >2 sync waits; split
    extra waits off Drain/EventSemaphore into preceding wait-only ctrls."""
    j = json.loads(raw)
    for fn in j["functions"]:
        for bb in fn["blocks"]:
            out = []
            for inst in bb["instructions"]:
                si = inst.get("sync_info")
                waits = (si or {}).get("on_wait") or []
                limit = _MAX_WAITS if inst.get("opcode") == "EventSemaphore" else 1
                if len(waits) > limit:
                    head, tail = waits[:-limit], waits[-limit:]
                    for k in range(0, len(head), _MAX_WAITS):
                        out.append({
                            "debug": inst.get("debug", 0),
                            "engine": inst["engine"],
                            "ins": [], "outs": [],
                            "name": inst["name"] + f"_w{k}",
                            "opcode": "EventSemaphore",
                            "sync_info": {"on_update": [], "on_wait": head[k:k + _MAX_WAITS]},
                        })
                    si["on_wait"] = tail
                out.append(inst)
            bb["instructions"] = out
    return json.dumps(j).encode()


def _patch_bass(nc):
    orig = nc.to_json_bytes
    nc.to_json_bytes = lambda: _split_waits_json(orig())
    return nc


# ---------------------------------------------------------------- the program
_cache = {}


def _build(is_causal: bool, use_bias: bool = True):
    import concourse.bass as bass
    import concourse.tile as tile
    import concourse.mybir as mybir
    from contextlib import ExitStack

    f32 = mybir.dt.float32
    f16 = mybir.dt.float16
    Alu = mybir.AluOpType
    Act = mybir.ActivationFunctionType

    nc = bass.Bass("TRN2", target_bir_lowering=False, debug=False, enable_asserts=False)

    xt = nc.dram_tensor("xt", [E, TPC], f16, kind="ExternalInput").ap()
    ws = {n: nc.dram_tensor(n, [E, E], f16, kind="ExternalInput").ap()
          for n in ("wq", "wk", "wv", "wo")}
    bias4 = nc.dram_tensor("bias4", [1, 4 * E], f16, kind="ExternalInput").ap()
    ident = nc.dram_tensor("ident", [128, 128], f16, kind="ExternalInput").ap()
    out_d = nc.dram_tensor("out", [TPC, E], f32, kind="ExternalOutput").ap()

    def tt(eng, out, in0, in1, op):
        return eng.add_instruction(mybir.InstTensorTensor(
            name=nc.get_next_instruction_name(), op=op,
            ins=[eng.lower_ap(in0), eng.lower_ap(in1)],
            outs=[eng.lower_ap(out)]))

    with tile.TileContext(nc) as tc, ExitStack() as ctx:
        wpool = ctx.enter_context(tc.tile_pool(name="w", bufs=1))
        xpool = ctx.enter_context(tc.tile_pool(name="x", bufs=2))
        qkv = ctx.enter_context(tc.tile_pool(name="qkv", bufs=3))
        gridp = ctx.enter_context(tc.tile_pool(name="grid", bufs=2))
        ppool = ctx.enter_context(tc.tile_pool(name="p", bufs=2))
        stats = ctx.enter_context(tc.tile_pool(name="st", bufs=2))
        qkd = ctx.enter_context(tc.tile_pool(name="qkd", bufs=1))
        pvp = ctx.enter_context(tc.tile_pool(name="pv", bufs=1))
        opool = ctx.enter_context(tc.tile_pool(name="o", bufs=3))
        aop = ctx.enter_context(tc.tile_pool(name="ao", bufs=3))
        o2pool = ctx.enter_context(tc.tile_pool(name="o2", bufs=3))
        psum = ctx.enter_context(tc.tile_pool(name="ps", bufs=2, space="PSUM"))
        pstr = ctx.enter_context(tc.tile_pool(name="pstr", bufs=2, space="PSUM"))
        pso = ctx.enter_context(tc.tile_pool(name="pso", bufs=2, space="PSUM"))

        # ---------------- resident constants
        w_sb = {}
        for n in ("wq", "wk", "wv", "wo"):
            t = wpool.tile([128, KC, E], f16, tag=f"w_{n}", name=f"w_{n}")
            wr = ws[n].rearrange("(k p) n -> p k n", p=128)
            for k in range(KC):
                eng = (nc.sync, nc.gpsimd, nc.scalar)[k % 3]
                eng.dma_start(t[:, k, :], wr[:, k, :])
            w_sb[n] = t
        if use_bias:
            b_sb = wpool.tile([1, 4 * E], f16, tag="bias")
            nc.sync.dma_start(b_sb[:], bias4[:])
            ones = wpool.tile([1, CHUNK], f16, tag="ones")
            nc.vector.memset(ones[:], 1.0)
        id_sb = wpool.tile([128, 128], f16, tag="ident")
        nc.sync.dma_start(id_sb[:], ident[:])

        xt_r = xt.rearrange("(k p) t -> p k t", p=128)

        # ---------------- per-chunk stage issue helpers
        state = {}

        def issue_proj(ci):
            tsl = slice(ci * CHUNK, (ci + 1) * CHUNK)
            x_sb = xpool.tile([128, KC, CHUNK], f16, tag="x")
            nc.sync.dma_start(x_sb[:], xt_r[:, :, tsl])
            sb = {}
            for pi, n in enumerate(("wq", "wk", "wv")):
                ps = psum.tile([128, E], f32, tag="proj")
                for k in range(KC):
                    for nn in range(2):
                        nsl = slice(nn * 512, (nn + 1) * 512)
                        nc.tensor.matmul(
                            ps[:, nsl],
                            x_sb[:, k, :],
                            w_sb[n][:, k, nsl],
                            start=(k == 0),
                            stop=(k == KC - 1 and not use_bias))
                if use_bias:
                    for nn in range(2):
                        nsl = slice(nn * 512, (nn + 1) * 512)
                        nc.tensor.matmul(
                            ps[:, nsl],
                            ones[:],
                            b_sb[0:1, pi * E + nn * 512: pi * E + (nn + 1) * 512],
                            start=False, stop=True)
                t = qkv.tile([128, E], f16, tag=n, name=n)
                nc.scalar.copy(t[:], ps[:])
                sb[n] = t
            state[ci] = {"q": sb["wq"], "k": sb["wk"], "v": sb["wv"],
                         "x": x_sb, "tsl": tsl}

        def issue_qk(ci):
            st = state[ci]
            q_sb, k_sb = st["q"], st["k"]
            grid = gridp.tile([128, H * H], f16, tag="grid")
            nc.gpsimd.memset(grid[:], NEG)
            k_v = k_sb[:].rearrange("p (g d) -> p g d", g=H)

            # --- DVE: all heads causal-packed: products + tree + finals
            gmax = (lambda h: h + 1) if is_causal else (lambda h: H)
            NP = sum(gmax(h) for h in range(H))
            off = [0] * (H + 1)
            for h in range(H):
                off[h + 1] = off[h] + gmax(h)
            qp_d = qkd.tile([128, NP * D], f16, tag="qpd")
            for h in range(H):
                g = gmax(h)
                tt(nc.vector,
                   qp_d[:, off[h] * D:off[h + 1] * D].rearrange("p (g d) -> p g d", g=g),
                   q_sb[:, h * D:(h + 1) * D].unsqueeze(1).broadcast_to([128, g, D]),
                   k_v[:, :g, :], Alu.mult)
            lv = qp_d[:].rearrange("p (n d) -> p n d", n=NP)
            w = D
            li = 0
            while w > 2:
                w //= 2
                nt = qkd.tile([128, NP * w], f16, tag=f"dtr{li}", name=f"dtr{li}")
                tt(nc.vector,
                   nt[:].rearrange("p (n d) -> p n d", n=NP),
                   lv[:, :, 0:w], lv[:, :, w:2 * w], Alu.add)
                lv = nt[:].rearrange("p (n d) -> p n d", n=NP)
                li += 1
            for h in range(H):
                g = gmax(h)
                tt(nc.vector,
                   grid[:, h * H:h * H + g].unsqueeze(2),
                   lv[:, off[h]:off[h + 1], 0:1], lv[:, off[h]:off[h + 1], 1:2],
                   Alu.add)
            st["grid"] = grid

        def issue_softmax(ci):
            st = state[ci]
            grid = st["grid"]
            g3 = grid[:].rearrange("p (h g) -> p h g", h=H)
            mx = stats.tile([128, H], f16, tag="mx")
            nc.vector.tensor_reduce(mx[:], g3, mybir.AxisListType.X, Alu.max)
            p2 = ppool.tile([128, H * H], f16, tag="psub")
            tt(nc.vector,
               p2[:].rearrange("p (h g) -> p h g", h=H),
               g3, mx[:].unsqueeze(2).broadcast_to([128, H, H]), Alu.subtract)
            ex = ppool.tile([128, H * H], f16, tag="pexp")
            nc.scalar.activation(ex[:], p2[:], Act.Exp, scale=float(SCALE))
            sm = stats.tile([128, H], f32, tag="sm")
            nc.vector.tensor_reduce(
                sm[:], ex[:].rearrange("p (h g) -> p h g", h=H),
                mybir.AxisListType.X, Alu.add)
            rc = stats.tile([128, H], f32, tag="rc")
            nc.vector.reciprocal(rc[:], sm[:])
            rc16 = stats.tile([128, H], f16, tag="rc16")
            nc.scalar.copy(rc16[:], rc[:])
            pn = ppool.tile([128, H * H], f16, tag="pnorm")
            tt(nc.vector,
               pn[:].rearrange("p (h g) -> p h g", h=H),
               ex[:].rearrange("p (h g) -> p h g", h=H),
               rc16[:].unsqueeze(2).broadcast_to([128, H, H]), Alu.mult)
            st["pn"] = pn

        def _pv_group(eng, st, h0, h1, Gp, tagc):
            pn, v_sb = st["pn"], st["v"]
            attn = st["attn"]
            v_dg = v_sb[:].rearrange("p (d g) -> p d g", g=H)  # V is d-major
            nh = h1 - h0
            gp = pvp.tile([128, nh * D * Gp], f16, tag=f"gp{tagc}", name=f"gp{tagc}")
            tt(eng,
               gp[:].rearrange("p (h d g) -> p h d g", h=nh, d=D),
               pn[:, h0 * H:h1 * H]
               .rearrange("p (h g) -> p h g", h=nh)[:, :, :Gp]
               .unsqueeze(2).broadcast_to([128, nh, D, Gp]),
               v_dg[:, :, :Gp].unsqueeze(1)
               .broadcast_to([128, nh, D, Gp]),
               Alu.mult)
            lvv = gp[:].rearrange("p (n g) -> p n g", n=nh * D)
            w2 = Gp
            li = 0
            while w2 > 2:
                w2 //= 2
                nt2 = pvp.tile([128, nh * D * w2], f16,
                               tag=f"pt{tagc}{li}", name=f"pt{tagc}{li}")
                tt(eng,
                   nt2[:].rearrange("p (n g) -> p n g", n=nh * D),
                   lvv[:, :, 0:w2], lvv[:, :, w2:2 * w2], Alu.add)
                lvv = nt2[:].rearrange("p (n g) -> p n g", n=nh * D)
                li += 1
            tt(eng,
               attn[:, h0 * D:h1 * D].unsqueeze(2),
               lvv[:, :, 0:1], lvv[:, :, 1:2], Alu.add)

        def issue_pv(ci):
            st = state[ci]
            attn = opool.tile([128, E], f16, tag="attn")
            st["attn"] = attn
            with nc.allow_low_precision("fp16 attn accumulation over 16 heads"):
                if is_causal:
                    _pv_group(nc.gpsimd, st, 0, 4, 4, "a")
                    _pv_group(nc.vector, st, 4, 8, 8, "b")
                    _pv_group(nc.vector, st, 8, 16, 16, "c")
                else:
                    _pv_group(nc.gpsimd, st, 0, 2, 16, "n")
                    _pv_group(nc.vector, st, 2, 16, 16, "m")

        def issue_out(ci):
            st = state[ci]
            attn, tsl = st["attn"], st["tsl"]
            ps_t = pstr.tile([128, E], f16, tag="tr")
            for j in range(KC):
                nc.tensor.transpose(
                    ps_t[:, j * 128:(j + 1) * 128],
                    attn[:, j * 128:(j + 1) * 128], id_sb[:])
            ao = aop.tile([128, KC, CHUNK], f16, tag="ao")
            nc.scalar.copy(ao[:], ps_t[:])
            o_sb = o2pool.tile([128, E], f32, tag="out", name="o_sb")
            for nn in range(2):
                nsl = slice(nn * 512, (nn + 1) * 512)
                ps_o = pso.tile([128, 512], f32, tag="oproj")
                for k in range(KC):
                    nc.tensor.matmul(
                        ps_o[:],
                        ao[:, k, :],
                        w_sb["wo"][:, k, nsl],
                        start=(k == 0),
                        stop=(k == KC - 1 and not use_bias))
                if use_bias:
                    nc.tensor.matmul(
                        ps_o[:],
                        ones[:],
                        b_sb[0:1, 3 * E + nn * 512: 3 * E + (nn + 1) * 512],
                        start=False, stop=True)
                nc.scalar.copy(o_sb[:, nsl], ps_o[:])
            nc.sync.dma_start(out_d[tsl, :], o_sb[:])
            del state[ci]

        # ---------------- software-pipelined issue order
        # softmax/pv(ci-1) first: their DVE/ACT/Pool ops are ready at the top
        # of the iteration; proj(ci) then refills PE; qk(ci) lands when the
        # q/k copies arrive; out(ci-1) uses PE after proj(ci).
        for ci in range(NCH + 1):
            if ci >= 1:
                issue_softmax(ci - 1)
                issue_pv(ci - 1)
            if ci < NCH:
                issue_proj(ci)
                issue_qk(ci)
            if ci >= 1:
                issue_out(ci - 1)

    return _patch_bass(nc)


LAST_RESULTS = None


def kernel(**inputs) -> np.ndarray:
    global LAST_RESULTS
    from concourse import bass_utils

    qkv = np.asarray(inputs["QKV"], dtype=np.float32)
    is_causal = bool(int(np.asarray(inputs["is_causal"])))
    X = np.ascontiguousarray(qkv.reshape(TOK, E).astype(np.float16))
    wts = {n: np.ascontiguousarray(np.asarray(inputs[wn], dtype=np.float32).T.astype(np.float16))
           for n, wn in (("wq", "Wq"), ("wk", "Wk"), ("wv", "Wv"), ("wo", "Wo"))}
    # V projection emits d-major head layout: col d*16+g holds head g, dim d
    wts["wv"] = np.ascontiguousarray(
        wts["wv"].reshape(E, H, D).transpose(0, 2, 1).reshape(E, E))
    bias4 = np.ascontiguousarray(np.concatenate([
        np.asarray(inputs[b], dtype=np.float32) for b in ("bq", "bk", "bv", "bo")])[None, :].astype(np.float16))
    ident = np.eye(128, dtype=np.float16)

    use_bias = any(
        float(np.abs(np.asarray(inputs[b])).max()) != 0.0
        for b in ("bq", "bk", "bv", "bo"))
    key = (is_causal, use_bias)
    if key not in _cache:
        _cache[key] = _build(is_causal, use_bias)
    nc = _cache[key]

    in_maps = []
    for c in range(NCORES):
        xt_c = np.ascontiguousarray(X[c * TPC:(c + 1) * TPC].T)
        in_maps.append({"xt": xt_c, "bias4": bias4, "ident": ident, **wts})

    trace = bool(int(os.environ.get("BASSMHA_TRACE", "0")))
    res = bass_utils.run_bass_kernel_spmd(
        nc, in_maps, core_ids=list(range(NCORES)), trace=trace)
    LAST_RESULTS = res
    out = np.concatenate([res.results[c]["out"] for c in range(NCORES)], axis=0)
    return out.reshape(B, S, E)


if __name__ == "__main__":
    np.random.seed(0)
    fake = {
        "QKV": np.random.randn(B, S, E).astype(np.float32),
        "Wq": np.random.randn(E, E).astype(np.float32) * 0.02,
        "bq": np.zeros(E, np.float32),
        "Wk": np.random.randn(E, E).astype(np.float32) * 0.02,
        "bk": np.zeros(E, np.float32),
        "Wv": np.random.randn(E, E).astype(np.float32) * 0.02,
        "bv": np.zeros(E, np.float32),
        "Wo": np.random.randn(E, E).astype(np.float32) * 0.02,
        "bo": np.zeros(E, np.float32),
        "is_causal": 1,
    }
    o = kernel(**fake)
    print("kernel ok", o.shape, o.dtype, float(np.abs(o).mean()))
